# revision 21
# baseline (speedup 1.0000x reference)
"""Trainium2 Bass kernel for DeepgazeSpadeV2 segment_reduce.

Computes, for feats [B=2, C=768, 18, 18] and segmap [B=2, 256, 256] (S=256):
  1. nearest-downsample segmap to 18x18 patch segment ids
  2. scatter-mean patch features into a per-batch [S, C] table
  3. paint: out[b, :, y, x] = table_b[segmap[b, y, x], :]  -> [B, C, 256, 256]

Sharding: 8 cores = 2 batches x 4 row-slices of the output image. Each core
rebuilds its batch's (tiny) segment table and paints its 64-row slice.

On-device algorithm (per core), all-bf16 datapath (fp32 PSUM accumulate):
  - scatter:  sums[s, c] = onehot_patch[p, s]^T @ featsT[p, c]
              counts[s]  = onehot_patch^T @ ones    (exact: 0/1 in bf16)
              table = sums * (1 / max(counts, 1))   -> bf16 [S, C]
  - paint:    out[c_tile, pix] = sum_s table[s, c] * onehot_pix[s, pix] via
              normal bf16 matmuls (1 cyc/col, 2x faster than the fp32
              transpose-mode gather). onehot_pix[s, pix] = (segmap[pix] == s)
              built on GpSimd (is_equal) from a HOST-replicated [128, NPIX]
              segment-id map DMA'd straight into SBUF (the on-device
              partition_broadcast chain cost ~16us of serial startup and
              contended with DVE). Output downcast to bf16 in the PSUM ->
              SBUF stage (ACT/DVE copies 1:1), then 256KB HWDGE DMAs; the
              host upcasts to fp32 (bf16 rounding ~0.4%% max rel err vs the
              2e-2 gate). A K=128 PE warmup burst trips the HAM clock gate
              before the scatter so phase A runs at 2.4GHz.
"""

import sys

if "/opt/trn_rl_repo" not in sys.path:
    sys.path.insert(0, "/opt/trn_rl_repo")

import numpy as np
import ml_dtypes

B, C, HP, WP = 2, 768, 18, 18
HI, WI = 256, 256
S = 256
NP_PATCH = HP * WP            # 324
P_PAD = 384                   # 3 x 128 partition chunks
N_CORES = 8
SLICES_PER_BATCH = N_CORES // B
ROWS_PER_SLICE = HI // SLICES_PER_BATCH   # 64
NPIX = ROWS_PER_SLICE * WI                # 16384
BLK = 512                                 # pixels per PSUM block
NBLK = NPIX // BLK                        # 32
SUPER = 4                                 # blocks per staged output DMA
CT = C // 128                             # 6 channel tiles

_CACHE = {}


def _build():
    import concourse.bacc as bacc
    import concourse.mybir as mybir
    from concourse.tile import TileContext

    f32 = mybir.dt.float32
    bf16 = mybir.dt.bfloat16
    EQ = mybir.AluOpType.is_equal
    MULT = mybir.AluOpType.mult

    nc = bacc.Bacc("TRN2", target_bir_lowering=False, debug=False)
    featsT = nc.dram_tensor("featsT", [P_PAD, C], bf16, kind="ExternalInput")
    segp = nc.dram_tensor("segp", [P_PAD], f32, kind="ExternalInput")
    u8 = mybir.dt.uint8
    segb = nc.dram_tensor("segb", [128, NPIX], bf16, kind="ExternalInput")
    out = nc.dram_tensor("out", [C, NPIX], bf16, kind="ExternalOutput")

    with TileContext(nc) as tc:
        with (
            tc.tile_pool(name="const", bufs=1) as cp,
            tc.tile_pool(name="work", bufs=3) as wp,
            tc.tile_pool(name="stage", bufs=2) as sp,
        ):
            # ---- phase A: build the segment table ----
            # PE warmup burst: trip the HAM clock gate (4096-cycle activity
            # window) before the scatter matmuls arrive, so they run at 2.4GHz
            psA_cm0 = tc.tile_pool(name="psW", bufs=1, space="PSUM")
            psW = psA_cm0.__enter__()
            warm_w = cp.tile([128, 64], bf16, tag="warm_w")
            nc.any.memset(warm_w[:, :], 1.0)
            warm_x = cp.tile([128, 64], bf16, tag="warm_x")
            nc.any.memset(warm_x[:, :], 1.0)
            ps_warm = psW.tile([64, 64], f32, tag="warm")
            NWARM = 24
            for i in range(NWARM):
                nc.tensor.matmul(
                    ps_warm[:, :], warm_w[:, :], warm_x[:, :],
                    start=(i == 0), stop=(i == NWARM - 1),
                )
            psA_cm0.__exit__(None, None, None)

            sp_f = cp.tile([128, 3], f32, tag="sp_f")
            nc.sync.dma_start(out=sp_f[:, :], in_=segp.ap().rearrange("(k p) -> p k", p=128))
            ft = cp.tile([128, 3, C], bf16, tag="ft")
            ftr = featsT.ap().rearrange("(k p) c -> p k c", p=128)
            for k in range(3):
                nc.sync.dma_start(out=ft[:, k, :], in_=ftr[:, k, :])

            # pre-replicated full-res segment ids: [128, NPIX] bf16, chunked
            # so the first superblocks' one-hots can start early
            sgb_all = cp.tile([128, NPIX], bf16, tag="sgb_all")
            SGCHUNK = 2048
            for cch in range(NPIX // SGCHUNK):
                nc.sync.dma_start(
                    out=sgb_all[:, cch * SGCHUNK : (cch + 1) * SGCHUNK],
                    in_=segb.ap()[:, cch * SGCHUNK : (cch + 1) * SGCHUNK],
                )

            io_f = cp.tile([128, S], bf16, tag="io_f")
            nc.gpsimd.iota(io_f[:, :], pattern=[[1, S]], base=0, channel_multiplier=0,
                           allow_small_or_imprecise_dtypes=True)
            io2_f = cp.tile([128, 2], f32, tag="io2_f")
            nc.gpsimd.iota(io2_f[:, :], pattern=[[128, 2]], base=0, channel_multiplier=1,
                           allow_small_or_imprecise_dtypes=True)

            ones_col = cp.tile([128, 1], bf16, tag="ones_col")
            nc.any.memset(ones_col[:, :], 1.0)

            # one-hot over patches: ohp[p, k, s] = (segp[k*128+p] == s)
            ohp = cp.tile([128, 3, S], bf16, tag="ohp")
            for k in range(3):
                nc.vector.tensor_scalar(ohp[:, k, :], io_f[:, :], sp_f[:, k : k + 1], None, EQ)

            # counts / recip per s-tile
            psA_cm = tc.tile_pool(name="psA", bufs=2, space="PSUM")
            psA = psA_cm.__enter__()
            recip = cp.tile([128, 2], f32, tag="recip")
            for st in range(2):
                ps_cnt = psA.tile([128, 1], f32, tag="cnt")
                for k in range(3):
                    nc.tensor.matmul(
                        ps_cnt[:, :],
                        ohp[:, k, st * 128 : (st + 1) * 128],
                        ones_col[:, :],
                        start=(k == 0),
                        stop=(k == 2),
                    )
                cnt_cl = wp.tile([128, 1], f32, tag="cnt_cl")
                nc.vector.tensor_scalar_max(cnt_cl[:, :], ps_cnt[:, :], 1.0)
                nc.vector.reciprocal(recip[:, st : st + 1], cnt_cl[:, :])

            # sums and mean, in [s, c] layout (2 s-tiles x 2 c-chunks of 384)
            tab = cp.tile([128, 2, C], bf16, tag="tab")
            for st in range(2):
                for cc in range(2):
                    ps_sum = psA.tile([128, 384], f32, tag="sums")
                    for k in range(3):
                        nc.tensor.matmul(
                            ps_sum[:, :],
                            ohp[:, k, st * 128 : (st + 1) * 128],
                            ft[:, k, cc * 384 : (cc + 1) * 384],
                            start=(k == 0),
                            stop=(k == 2),
                        )
                    nc.vector.tensor_scalar(
                        tab[:, st, cc * 384 : (cc + 1) * 384],
                        ps_sum[:, :],
                        recip[:, st : st + 1],
                        None,
                        MULT,
                    )

            psA_cm.__exit__(None, None, None)

            # ---- phase B: paint ----
            copy_flip = [0]
            with tc.tile_pool(name="psB", bufs=4, space="PSUM") as psB:
                for sb in range(NBLK // SUPER):
                    sblk = SUPER * BLK
                    # one-hot tiles for the blocks of this superblock
                    ohs = []
                    for j in range(SUPER):
                        oh = wp.tile([128, 2, BLK], bf16, tag="oh", bufs=6, name="oh")
                        for st in range(2):
                            nc.vector.tensor_scalar(
                                oh[:, st, :],
                                sgb_all[:, sb * sblk + j * BLK : sb * sblk + (j + 1) * BLK],
                                io2_f[:, st : st + 1],
                                None,
                                EQ,
                            )
                        ohs.append(oh)
                    stages = [
                        sp.tile([128, sblk], bf16, tag=f"stg{ct}", name=f"stg{ct}")
                        for ct in range(CT)
                    ]
                    for ct in range(CT):
                        for half in range(SUPER // 2):
                            ps_o = psB.tile([128, 2 * BLK], f32, tag="out")
                            for k in range(2):
                                for j in range(2):
                                    oh = ohs[half * 2 + j]
                                    dst = ps_o[:, j * BLK : (j + 1) * BLK]
                                    nc.tensor.matmul(
                                        dst, tab[:, k, ct * 128 : (ct + 1) * 128],
                                        oh[:, k, :],
                                        start=(k == 0), stop=(k == 1),
                                    )
                            dst_stage = stages[ct][:, half * 2 * BLK : (half + 1) * 2 * BLK]
                            if copy_flip[0] % 5 < 3:
                                nc.scalar.copy(out=dst_stage, in_=ps_o[:, :])
                            else:
                                nc.vector.tensor_copy(dst_stage, ps_o[:, :])
                            copy_flip[0] += 1
                    for ct in range(CT):
                        nc.sync.dma_start(
                            out=out.ap()[
                                ct * 128 : (ct + 1) * 128,
                                sb * sblk : (sb + 1) * sblk,
                            ],
                            in_=stages[ct][:, :],
                        )
    nc.compile()
    return nc


def _get_nc():
    if "nc" not in _CACHE:
        _CACHE["nc"] = _build()
    return _CACHE["nc"]


def _make_in_maps(feats, segmap):
    idx_h = (np.arange(HP) * HI) // HP
    idx_w = (np.arange(WP) * WI) // WP
    in_maps = []
    for core in range(N_CORES):
        b = core // SLICES_PER_BATCH
        q = core % SLICES_PER_BATCH
        ftp = np.zeros((P_PAD, C), dtype=ml_dtypes.bfloat16)
        ftp[:NP_PATCH] = feats[b].reshape(C, NP_PATCH).T.astype(ml_dtypes.bfloat16)
        spp = np.full((P_PAD,), S, dtype=np.float32)  # pad matches no segment
        seg_b = np.clip(segmap[b], 0, S - 1)  # reference clips ids to [0, S-1]
        spp[:NP_PATCH] = seg_b[idx_h[:, None], idx_w[None, :]].reshape(-1).astype(np.float32)
        pix = seg_b[q * ROWS_PER_SLICE : (q + 1) * ROWS_PER_SLICE, :].reshape(-1)
        pixb = np.ascontiguousarray(
            np.broadcast_to(pix.astype(ml_dtypes.bfloat16)[None, :], (128, NPIX))
        )
        in_maps.append(
            {
                "featsT": ftp,
                "segp": spp,
                "segb": pixb,
            }
        )
    return in_maps


def _run(in_maps, **kwargs):
    from concourse.bass_utils import run_bass_kernel_spmd

    nc = _get_nc()
    return run_bass_kernel_spmd(nc, in_maps, core_ids=list(range(N_CORES)), **kwargs)


def kernel(feats, segmap, num_total_segments):
    feats = np.asarray(feats, dtype=np.float32)
    segmap = np.asarray(segmap, dtype=np.int32)
    assert int(num_total_segments) == S
    assert feats.shape == (B, C, HP, WP) and segmap.shape == (B, HI, WI)

    res = _run(_make_in_maps(feats, segmap))
    out = np.empty((B, C, HI, WI), dtype=np.float32)
    for core in range(N_CORES):
        b = core // SLICES_PER_BATCH
        q = core % SLICES_PER_BATCH
        out[b, :, q * ROWS_PER_SLICE : (q + 1) * ROWS_PER_SLICE, :] = (
            res.results[core]["out"].astype(np.float32).reshape(C, ROWS_PER_SLICE, WI)
        )
    return out
